# revision 7
# baseline (speedup 1.0000x reference)
"""Trainium2 Bass kernel for AlignedQuestionEmbeddingLayer.

Computation (per batch element):
    C = relu(Xc @ W.T + b)            # [4096, 128]
    Q = relu(Xq @ W.T + b)            # [512, 128]
    S = C @ Q.T  (+ mask)             # [4096, 512]
    A = softmax(S, axis=-1)
    out = A @ Q                       # [4096, 128]

Sharding: data-parallel over batch, one batch element per NeuronCore (8 cores).

Device-side design notes (v2):
  - Dense contracts over E=300 (+1 bias row) = chunks of 128/128/45 on the
    partition axis; the 45-row chunk is NOT zero-padded to 128, cutting
    input DMA bytes by ~21%. Row 300 of xT is all-ones and row 300 of the
    W blob holds b, so x@W.T+b comes straight out of the matmul.
  - Inputs are pre-transposed and fp16-cast on host (fp16 matmuls are 4x
    fp32 on the PE; no on-device transposes; half the DMA bytes).
  - The main loop runs 4 iterations of 1024 context rows (2x the old
    super-tile): half the DMA triggers (the Sync sequencer's ~0.6us
    serial cost per trigger), half the loop-carried semaphores.
  - bw/bq/xk2 setup DMAs issue on the Scalar (ACT) HWDGE queue in
    parallel with the x loads on Sync, so the question dense no longer
    waits ~7us for its operands (the old kernel lost ~2us + a HAM
    re-throttle there).
  - Scores are computed transposed ([q partitions, c free]) so the final
    matmul (lhsT=expS_T chunk, rhs=[Q|ones] in bf16) produces output rows
    and softmax denominators in one PSUM accumulation.
  - Softmax skips max-subtraction: scores are bounded (~|s|<40) so exp()
    is safe in fp32; exp is stored bf16 (fp16 lacks the range).
  - Output is stored fp16 (halves store traffic; |out| <= max|Q| so fp16
    is safe, ~5e-4 rel err) and upcast to fp32 on host.
  - The loop is software-pipelined with lag 1 (back/norm/store of it-1
    between dense and scores of it) and x loads run 2 iterations ahead.
  - A short burst of dummy matmuls at kernel start warms the PE HAM
    clock gate during the setup-DMA window (2.4 GHz vs 1.2 cold).
"""

import sys

import numpy as np

sys.path.insert(0, "/opt/trn_rl_repo")

B, CTX, QST, E, H = 8, 4096, 512, 300, 128
N_CORES = 8
EA = E + 1          # augmented row E carries the bias / ones
K2P = EA - 256      # 45 partitions in the third contraction chunk
ST2 = 1024          # context rows per main-loop iteration
N_IT = CTX // ST2   # 4 iterations
N_WARM = 3          # dummy matmuls to warm the HAM clock gate

# fp16 blobs: W chunks (with bias row) and XqT chunks (with ones row)
BW_F = 3 * 128            # bw: [:, k*128 : +128], k=0..2 (k=2 rows 0:45)
BQ_F = 3 * 512            # bq: [:, k*512 : +512], k=0..2 (k=2 rows 0:45)
# fp32 blob: bias column + 4 maskbias columns (only read by masked variant)
OFF_B = 0
OFF_MB = 1
BLOB32_F = 5

_COMPILED = {}


def _build_kernel(n_it=N_IT, masked=False):
    import concourse.bass as bass
    import concourse.tile as tile
    from concourse import bacc, mybir

    f32 = mybir.dt.float32
    f16 = mybir.dt.float16
    bf16 = mybir.dt.bfloat16
    AF = mybir.ActivationFunctionType
    MAX = mybir.AluOpType.max

    nc = bacc.Bacc(
        "TRN2", target_bir_lowering=False, debug=False, num_devices=N_CORES
    )

    xk01_d = nc.declare_dram_parameter("xk01", [N_IT, 128, 2, ST2], f16,
                                       isOutput=False)
    xk2_d = nc.declare_dram_parameter("xk2", [K2P, CTX], f16, isOutput=False)
    bw_d = nc.declare_dram_parameter("bw", [128, BW_F], f16, isOutput=False)
    bq_d = nc.declare_dram_parameter("bq", [128, BQ_F], f16, isOutput=False)
    b32_d = nc.declare_dram_parameter("b32", [128, BLOB32_F], f32, isOutput=False)
    out_d = nc.declare_dram_parameter("out4", [N_IT, 128, ST2 // 128, H], f16,
                                      isOutput=True)

    with tile.TileContext(nc) as tc:
        with (
            tc.tile_pool(name="const", bufs=1) as const_pool,
            tc.tile_pool(name="xin", bufs=3) as xin_pool,
            tc.tile_pool(name="ct", bufs=2) as ct_pool,
            tc.tile_pool(name="exps", bufs=2) as exps_pool,
            tc.tile_pool(name="outs", bufs=2) as outs_pool,
            tc.tile_pool(name="pct", bufs=1, space=bass.MemorySpace.PSUM) as pct_pool,
            tc.tile_pool(name="pst", bufs=2, space=bass.MemorySpace.PSUM) as pst_pool,
            tc.tile_pool(name="po", bufs=2, space=bass.MemorySpace.PSUM) as po_pool,
        ):
            # ---- setup DMAs first: x0 (split per chunk) on Sync, the
            # shared blobs on the Scalar HWDGE queue in parallel ----------
            xk2_sb = const_pool.tile([K2P, CTX], f16, tag="xk2")

            def load_xa01(it, split=False):
                xa = xin_pool.tile([128, 2, ST2], f16, tag="xa")
                if split:
                    for k in range(2):
                        nc.sync.dma_start(xa[:, k, :], xk01_d[it, :, k, :])
                else:
                    nc.sync.dma_start(xa[:], xk01_d[it])
                return xa

            xas = {0: load_xa01(0, split=True)}
            bw_sb = const_pool.tile([128, BW_F], f16, tag="bw")
            nc.scalar.dma_start(bw_sb[:], bw_d[:])
            nc.scalar.dma_start(xk2_sb[:, 0:ST2], xk2_d[:, 0:ST2])
            bq_sb = const_pool.tile([128, BQ_F], f16, tag="bq")
            nc.scalar.dma_start(bq_sb[:], bq_d[:])
            nc.scalar.dma_start(xk2_sb[:, ST2:], xk2_d[:, ST2:])
            if masked:
                setup32 = const_pool.tile([128, BLOB32_F], f32, tag="setup32")
                nc.scalar.dma_start(setup32[:], b32_d[:])
            for _it in range(1, min(3, n_it)):
                xas[_it] = load_xa01(_it)

            # ---- PE warmup: matmuls on an uninitialized tile, results
            # discarded (next dense's start=True clears the bank) ----------
            warm = const_pool.tile([128, 512], f16, tag="warm")
            nc.gpsimd.memset(warm[:], 0.0)
            warm_ps = pct_pool.tile([H, ST2], f32, tag="pct")
            for _ in range(N_WARM):
                nc.tensor.matmul(
                    warm_ps[:, 0:512], warm[:, 0:128], warm[:], start=True,
                    stop=True, skip_group_check=True,
                )

            def w_chunk(k):  # [*, 128] fp16 W.T chunk (row E holds b)
                if k < 2:
                    return bw_sb[:, k * 128 : (k + 1) * 128]
                return bw_sb[0:K2P, 256:384]

            def q_chunk(k):  # [*, 512] fp16 Xq.T chunk (row E all-ones)
                if k < 2:
                    return bq_sb[:, k * 512 : (k + 1) * 512]
                return bq_sb[0:K2P, 1024:1536]

            def x_chunk(xa, it, k):
                if k < 2:
                    return xa[:, k, :]
                return xk2_sb[:, it * ST2 : (it + 1) * ST2]

            # question dense twice (bias rides the aug row):
            #   QT [h, q] fp16 for the scores lhsT
            #   Q  [q, h] bf16 (as [Q|ones] per chunk) for the final rhs
            qt_sb = const_pool.tile([H, QST], f16, tag="qt")
            qa_sb = []
            for j in range(4):
                qa_tile = const_pool.tile([128, H + 1], bf16, tag=f"qa{j}")
                qa_sb.append(qa_tile)

            def qt_setup():
                psum_q = pst_pool.tile([128, ST2], f32, tag="pst")
                for k in range(3):
                    nc.tensor.matmul(
                        psum_q[0:H, 0:QST], w_chunk(k), q_chunk(k),
                        start=(k == 0), stop=(k == 2),
                    )
                nc.vector.tensor_scalar(qt_sb[:], psum_q[0:H, 0:QST],
                                        0.0, None, MAX)

            def qa_setup():
                psum_qd = pst_pool.tile([128, ST2], f32, tag="pst")
                for j in range(4):
                    for k in range(3):
                        nc.tensor.matmul(
                            psum_qd[:, j * 128 : (j + 1) * 128],
                            q_chunk(k)[:, j * 128 : (j + 1) * 128], w_chunk(k),
                            start=(k == 0), stop=(k == 2),
                        )
                for j in range(4):
                    nc.vector.tensor_scalar(
                        qa_sb[j][:, 0:H], psum_qd[:, j * 128 : (j + 1) * 128],
                        0.0, None, MAX,
                    )
                    nc.gpsimd.memset(qa_sb[j][:, H : H + 1], 1.0)

            # ---- software-pipelined main loop -----------------------------
            def dense_phase(it, xa):
                # one matmul may only target a single PSUM bank (512 fp32
                # free elements), so each k-chunk runs as two halves
                psum_ct = pct_pool.tile([128, ST2], f32, tag="pct")
                for h in range(2):
                    cols = slice(h * 512, (h + 1) * 512)
                    for k in range(3):
                        nc.tensor.matmul(
                            psum_ct[0:H, cols], w_chunk(k),
                            x_chunk(xa, it, k)[:, cols],
                            start=(k == 0), stop=(k == 2),
                        )
                ct_sb = ct_pool.tile([H, ST2], f16, tag="ct")
                nc.vector.tensor_scalar(ct_sb[:], psum_ct[0:H, :],
                                        0.0, None, MAX)
                return ct_sb

            def scores_exp_phase(ct_sb):
                es = exps_pool.tile([128, 4 * ST2], bf16, tag="es")
                for j in range(4):
                    ps = pst_pool.tile([128, ST2], f32, tag="pst")
                    for h in range(2):
                        cols = slice(h * 512, (h + 1) * 512)
                        nc.tensor.matmul(
                            ps[:, cols], qt_sb[:, j * 128 : (j + 1) * 128],
                            ct_sb[:, cols], start=True, stop=True,
                        )
                    eslice = es[:, j * ST2 : (j + 1) * ST2]
                    if masked:
                        nc.scalar.activation(
                            eslice, ps[:], AF.Exp,
                            bias=setup32[:, OFF_MB + j : OFF_MB + j + 1],
                        )
                    else:
                        nc.scalar.activation(eslice, ps[:], AF.Exp)
                return es

            def back_phase(it, es):
                def exp_chunk(ci, j):  # [128 q, 128 c] bf16 lhsT
                    return es[:, j * ST2 + ci * 128 : j * ST2 + (ci + 1) * 128]

                split_store = it == n_it - 1  # shorter kernel tail
                o_big = outs_pool.tile([128, ST2 // 128, H], f16, tag="obig")
                for cp in range(ST2 // 256):  # ci pairs share one PSUM bank
                    po2 = po_pool.tile([128, 2, H + 1], f32, tag="po")
                    for ch in range(2):
                        ci = 2 * cp + ch
                        for j in range(4):
                            nc.tensor.matmul(
                                po2[:, ch, :], exp_chunk(ci, j), qa_sb[j][:],
                                start=(j == 0), stop=(j == 3),
                            )
                    recip2 = outs_pool.tile([128, 2], f32, tag="recip")
                    sums2 = po2[:, :, H : H + 1].rearrange("p a b -> p (a b)")
                    nc.vector.reciprocal(recip2[:], sums2)
                    for ch in range(2):
                        ci = 2 * cp + ch
                        nc.vector.tensor_scalar_mul(
                            o_big[:, ci, :], po2[:, ch, 0:H],
                            recip2[:, ch : ch + 1])
                    if split_store:
                        nc.sync.dma_start(
                            out_d[it, :, 2 * cp : 2 * cp + 2, :],
                            o_big[:, 2 * cp : 2 * cp + 2, :])
                if not split_store:
                    nc.sync.dma_start(out_d[it], o_big[:])

            # prologue: dense+scores of it 0 as soon as x0 arrives, with
            # the (final-only) Q/qa dense interleaved
            cts = {0: dense_phase(0, xas.pop(0))}
            qt_setup()
            qa_setup()
            if n_it > 1:
                cts[1] = dense_phase(1, xas.pop(1))
            prev_exp = scores_exp_phase(cts.pop(0))
            for it in range(1, n_it + 1):
                if it < n_it:
                    if it + 2 < n_it:
                        xas[it + 2] = load_xa01(it + 2)
                    if it + 1 < n_it:
                        cts[it + 1] = dense_phase(it + 1, xas.pop(it + 1))
                    back_phase(it - 1, prev_exp)
                    prev_exp = scores_exp_phase(cts.pop(it))
                else:
                    back_phase(it - 1, prev_exp)

    return nc


def _get_nc(masked=False):
    key = ("nc", masked)
    if key not in _COMPILED:
        nc = _build_kernel(masked=masked)
        nc.compile()
        nc.finalize()
        _COMPILED[key] = nc
    return _COMPILED[key]


def make_blobs(W, b, question_sequence_i, question_mask_i):
    """Pack per-core constants into the fp16 and fp32 setup blobs."""
    wTp = np.zeros((EA, H), np.float16)
    wTp[:E] = W.astype(np.float16).T
    wTp[E] = b.astype(np.float16)          # bias rides the aug row
    bw = np.zeros((128, BW_F), np.float16)
    bw[:, 0:128] = wTp[0:128]
    bw[:, 128:256] = wTp[128:256]
    bw[0:K2P, 256:384] = wTp[256:EA]

    qTp = np.zeros((EA, QST), np.float16)
    qTp[:E] = question_sequence_i.astype(np.float16).T
    qTp[E] = 1.0                           # ones row pairs with the bias row
    bq = np.zeros((128, BQ_F), np.float16)
    bq[:, 0:512] = qTp[0:128]
    bq[:, 512:1024] = qTp[128:256]
    bq[0:K2P, 1024:1536] = qTp[256:EA]

    b32 = np.zeros((128, BLOB32_F), np.float32)
    b32[:, OFF_B] = b.astype(np.float32)
    mb = np.where(question_mask_i == 0, np.float32(-1e30), np.float32(0.0))
    b32[:, OFF_MB : OFF_MB + 4] = mb.reshape(4, 128).T
    return bw, bq, b32


def make_in_maps(context_sequence, question_sequence, question_mask, W, b):
    in_maps = []
    for i in range(N_CORES):
        xcT = np.empty((EA, CTX), np.float16)
        xcT[:E] = context_sequence[i].T.astype(np.float16)
        xcT[E] = 1.0                       # ones row pairs with the bias row
        # partition-major tiling with >=2KB contiguous per-partition rows
        xk01 = np.ascontiguousarray(
            xcT[0:256].reshape(2, 128, N_IT, ST2).transpose(2, 1, 0, 3))
        xk2 = np.ascontiguousarray(xcT[256:EA])
        bw, bq, b32 = make_blobs(W, b, question_sequence[i], question_mask[i])
        in_maps.append({"xk01": xk01, "xk2": xk2, "bw": bw, "bq": bq,
                        "b32": b32})
    return in_maps


def assemble_out(res):
    outs = []
    for i in range(N_CORES):
        o4 = res.results[i]["out4"]        # [it, p, ci, h] fp16
        outs.append(o4.transpose(0, 2, 1, 3).reshape(CTX, H))
    return np.stack(outs, axis=0).astype(np.float32)


def kernel(context_sequence, question_sequence, question_mask, W, b):
    from concourse.bass_utils import run_bass_kernel_spmd

    masked = bool(np.any(np.asarray(question_mask) == 0))
    nc = _get_nc(masked=masked)
    in_maps = make_in_maps(
        context_sequence, question_sequence, question_mask, W, b)
    res = run_bass_kernel_spmd(nc, in_maps, core_ids=list(range(N_CORES)))
    return assemble_out(res)


# revision 21
# speedup vs baseline: 1.0105x; 1.0105x over previous
"""Trainium2 Bass kernel for AlignedQuestionEmbeddingLayer.

Computation (per batch element):
    C = relu(Xc @ W.T + b)            # [4096, 128]
    Q = relu(Xq @ W.T + b)            # [512, 128]
    S = C @ Q.T  (+ mask)             # [4096, 512]
    A = softmax(S, axis=-1)
    out = A @ Q                       # [4096, 128]

Sharding: data-parallel over batch, one batch element per NeuronCore (8 cores).

Device-side design notes (v2):
  - Dense contracts over E=300 (+1 bias row) = chunks of 128/128/45 on the
    partition axis; the 45-row chunk is NOT zero-padded to 128, cutting
    input DMA bytes by ~21%. Row 300 of xT is all-ones and row 300 of the
    W blob holds b, so x@W.T+b comes straight out of the matmul.
  - Inputs are pre-transposed and fp16-cast on host (fp16 matmuls are 4x
    fp32 on the PE; no on-device transposes; half the DMA bytes).
  - The main loop runs 4 iterations of 1024 context rows (2x the old
    super-tile): half the DMA triggers (the Sync sequencer's ~0.6us
    serial cost per trigger), half the loop-carried semaphores.
  - bw/bq/xk2 setup DMAs issue on the Scalar (ACT) HWDGE queue in
    parallel with the x loads on Sync, so the question dense no longer
    waits ~7us for its operands (the old kernel lost ~2us + a HAM
    re-throttle there).
  - Scores are computed transposed ([q partitions, c free]) so the final
    matmul (lhsT=expS_T chunk, rhs=[Q|ones] in bf16) produces output rows
    and softmax denominators in one PSUM accumulation.
  - Softmax skips max-subtraction: scores are bounded (~|s|<40) so exp()
    is safe in fp32; exp is stored bf16 (fp16 lacks the range).
  - Output is stored fp16 (halves store traffic; |out| <= max|Q| so fp16
    is safe, ~5e-4 rel err) and upcast to fp32 on host.
  - The loop is software-pipelined with lag 1 (back/norm/store of it-1
    between dense and scores of it) and x loads run 2 iterations ahead.
  - A short burst of dummy matmuls at kernel start warms the PE HAM
    clock gate during the setup-DMA window (2.4 GHz vs 1.2 cold).
"""

import sys

import numpy as np

sys.path.insert(0, "/opt/trn_rl_repo")

B, CTX, QST, E, H = 8, 4096, 512, 300, 128
N_CORES = 8
EA = E + 1          # augmented row E carries the bias / ones
K2P = EA - 256      # 45 partitions in the third contraction chunk
ST2 = 1024          # context rows per main-loop iteration
N_IT = CTX // ST2   # 4 iterations
N_WARM = 6          # dummy matmuls: ~3.6us contiguous PE busy flips the HAM
                    # clock gate to 2.4GHz (needs a fully-busy 3.4us window)

# fp16 blobs: W chunks (with bias row) and XqT chunks (with ones row)
BW_F = 3 * 128            # bw: [:, k*128 : +128], k=0..2 (k=2 rows 0:45)
BQ_F = 3 * 512            # bq: [:, k*512 : +512], k=0..2 (k=2 rows 0:45)
# fp32 blob: bias column + 4 maskbias columns (only read by masked variant)
OFF_B = 0
OFF_MB = 1
BLOB32_F = 5

_COMPILED = {}


def _build_kernel(n_it=N_IT, masked=False):
    import concourse.bass as bass
    import concourse.tile as tile
    from concourse import bacc, mybir

    f32 = mybir.dt.float32
    f16 = mybir.dt.float16
    bf16 = mybir.dt.bfloat16
    AF = mybir.ActivationFunctionType
    MAX = mybir.AluOpType.max

    nc = bacc.Bacc(
        "TRN2", target_bir_lowering=False, debug=False, num_devices=N_CORES
    )

    xk01_d = nc.declare_dram_parameter("xk01", [N_IT, 128, 2, ST2], f16,
                                       isOutput=False)
    xk2_d = nc.declare_dram_parameter("xk2", [K2P, CTX], f16, isOutput=False)
    bw_d = nc.declare_dram_parameter("bw", [128, BW_F], f16, isOutput=False)
    bq_d = nc.declare_dram_parameter("bq", [128, BQ_F], f16, isOutput=False)

    b32_d = nc.declare_dram_parameter("b32", [128, BLOB32_F], f32, isOutput=False)
    out_d = nc.declare_dram_parameter("out4", [N_IT, 128, ST2 // 128, H], f16,
                                      isOutput=True)

    with tile.TileContext(nc) as tc:
        with (
            tc.tile_pool(name="const", bufs=1) as const_pool,
            tc.tile_pool(name="xin", bufs=3) as xin_pool,
            tc.tile_pool(name="ct", bufs=2) as ct_pool,
            tc.tile_pool(name="exps", bufs=2) as exps_pool,
            tc.tile_pool(name="outs", bufs=2) as outs_pool,
            tc.tile_pool(name="pct", bufs=1, space=bass.MemorySpace.PSUM) as pct_pool,
            tc.tile_pool(name="pst", bufs=2, space=bass.MemorySpace.PSUM) as pst_pool,
            tc.tile_pool(name="po", bufs=2, space=bass.MemorySpace.PSUM) as po_pool,
        ):
            # ---- setup DMAs first: x0 (split per chunk) on Sync, the
            # shared blobs on the Scalar HWDGE queue in parallel ----------
            xk2_sb = const_pool.tile([K2P, CTX], f16, tag="xk2")

            def load_xa01(it, split=False):
                xa = xin_pool.tile([128, 2, ST2], f16, tag="xa")
                if split:
                    for k in range(2):
                        nc.sync.dma_start(xa[:, k, :], xk01_d[it, :, k, :])
                else:
                    nc.sync.dma_start(xa[:], xk01_d[it])
                return xa

            xas = {0: load_xa01(0, split=True)}
            bw_sb = const_pool.tile([128, BW_F], f16, tag="bw")
            nc.scalar.dma_start(bw_sb[:], bw_d[:])
            bq_sb = const_pool.tile([128, BQ_F], f16, tag="bq")
            nc.scalar.dma_start(bq_sb[:], bq_d[:])
            nc.scalar.dma_start(xk2_sb[:, 0:ST2], xk2_d[:, 0:ST2])
            nc.scalar.dma_start(xk2_sb[:, ST2:], xk2_d[:, ST2:])
            if masked:
                setup32 = const_pool.tile([128, BLOB32_F], f32, tag="setup32")
                nc.scalar.dma_start(setup32[:], b32_d[:])
            for _it in range(1, min(3, n_it)):
                xas[_it] = load_xa01(_it)

            # ---- PE warmup: matmuls on an uninitialized tile, results
            # discarded (next dense's start=True clears the bank) ----------
            warm = const_pool.tile([128, 512], f16, tag="warm")
            nc.gpsimd.memset(warm[:], 0.0)
            warm_ps = pct_pool.tile([H, ST2], f32, tag="pct")
            for _ in range(N_WARM):
                nc.tensor.matmul(
                    warm_ps[:, 0:512], warm[:, 0:128], warm[:], start=True,
                    stop=True, skip_group_check=True,
                )

            def w_chunk(k):  # [*, 128] fp16 W.T chunk (row E holds b)
                if k < 2:
                    return bw_sb[:, k * 128 : (k + 1) * 128]
                return bw_sb[0:K2P, 256:384]

            def q_chunk(k):  # [*, 512] fp16 Xq.T chunk (row E all-ones)
                if k < 2:
                    return bq_sb[:, k * 512 : (k + 1) * 512]
                return bq_sb[0:K2P, 1024:1536]

            def x_chunk(xa, it, k):
                if k < 2:
                    return xa[:, k, :]
                return xk2_sb[:, it * ST2 : (it + 1) * ST2]

            # question dense twice (bias rides the aug row):
            #   QT [h, q] fp16 for the scores lhsT
            #   Q  [q, h] bf16 (as [Q|ones] per chunk) for the final rhs
            qt_sb = const_pool.tile([H, QST], f16, tag="qt")
            qa_sb = []
            for j in range(4):
                qa_tile = const_pool.tile([128, H + 1], bf16, tag=f"qa{j}")
                qa_sb.append(qa_tile)

            def qt_setup():
                psum_q = pst_pool.tile([128, ST2], f32, tag="pst")
                for k in range(3):
                    nc.tensor.matmul(
                        psum_q[0:H, 0:QST], w_chunk(k), q_chunk(k),
                        start=(k == 0), stop=(k == 2),
                    )
                nc.vector.tensor_scalar(qt_sb[:], psum_q[0:H, 0:QST],
                                        0.0, None, MAX)

            def qa_setup():
                psum_qd = pst_pool.tile([128, ST2], f32, tag="pst")
                for j in range(4):
                    for k in range(3):
                        nc.tensor.matmul(
                            psum_qd[:, j * 128 : (j + 1) * 128],
                            q_chunk(k)[:, j * 128 : (j + 1) * 128], w_chunk(k),
                            start=(k == 0), stop=(k == 2),
                        )
                for j in range(4):
                    nc.vector.tensor_scalar(
                        qa_sb[j][:, 0:H], psum_qd[:, j * 128 : (j + 1) * 128],
                        0.0, None, MAX,
                    )
                    nc.gpsimd.memset(qa_sb[j][:, H : H + 1], 1.0)

            # ---- software-pipelined main loop -----------------------------
            def dense_phase(it, xa):
                # one matmul may only target a single PSUM bank (512 fp32
                # free elements), so each k-chunk runs as two halves
                psum_ct = pct_pool.tile([128, ST2], f32, tag="pct")
                for h in range(2):
                    cols = slice(h * 512, (h + 1) * 512)
                    for k in range(3):
                        nc.tensor.matmul(
                            psum_ct[0:H, cols], w_chunk(k),
                            x_chunk(xa, it, k)[:, cols],
                            start=(k == 0), stop=(k == 2),
                        )
                ct_sb = ct_pool.tile([H, ST2], f16, tag="ct")
                for h in range(2):  # halves so scores can start on half 0
                    cols = slice(h * 512, (h + 1) * 512)
                    nc.vector.tensor_scalar(ct_sb[:, cols], psum_ct[0:H, cols],
                                            0.0, None, MAX)
                return ct_sb

            def scores_exp_phase(ct_sb):
                es = exps_pool.tile([128, 4 * ST2], bf16, tag="es")
                for j in range(4):
                    ps = pst_pool.tile([128, ST2], f32, tag="pst")
                    for h in range(2):
                        cols = slice(h * 512, (h + 1) * 512)
                        nc.tensor.matmul(
                            ps[:, cols], qt_sb[:, j * 128 : (j + 1) * 128],
                            ct_sb[:, cols], start=True, stop=True,
                        )
                    eslice = es[:, j * ST2 : (j + 1) * ST2]
                    if masked:
                        nc.scalar.activation(
                            eslice, ps[:], AF.Exp,
                            bias=setup32[:, OFF_MB + j : OFF_MB + j + 1],
                        )
                    else:
                        nc.scalar.activation(eslice, ps[:], AF.Exp)
                return es

            def back_phase(it, es):
                def exp_chunk(ci, j):  # [128 q, 128 c] bf16 lhsT
                    return es[:, j * ST2 + ci * 128 : j * ST2 + (ci + 1) * 128]

                split_store = it == n_it - 1  # shorter kernel tail
                o_big = outs_pool.tile([128, ST2 // 128, H], f16, tag="obig")
                for cp in range(ST2 // 256):  # ci pairs share one PSUM bank
                    po2 = po_pool.tile([128, 2, H + 1], f32, tag="po")
                    for ch in range(2):
                        ci = 2 * cp + ch
                        for j in range(4):
                            nc.tensor.matmul(
                                po2[:, ch, :], exp_chunk(ci, j), qa_sb[j][:],
                                start=(j == 0), stop=(j == 3),
                            )
                    recip2 = outs_pool.tile([128, 2], f32, tag="recip")
                    sums2 = po2[:, :, H : H + 1].rearrange("p a b -> p (a b)")
                    nc.vector.reciprocal(recip2[:], sums2)
                    for ch in range(2):
                        ci = 2 * cp + ch
                        nc.vector.tensor_scalar_mul(
                            o_big[:, ci, :], po2[:, ch, 0:H],
                            recip2[:, ch : ch + 1])
                    if split_store and cp % 2 == 1:
                        nc.sync.dma_start(
                            out_d[it, :, 2 * cp - 2 : 2 * cp + 2, :],
                            o_big[:, 2 * cp - 2 : 2 * cp + 2, :])
                if not split_store:
                    nc.sync.dma_start(out_d[it], o_big[:])

            # prologue: dense+scores of it 0 as soon as x0 arrives; scores 0
            # goes before dense 1 so the exp chain on ACT starts earliest
            cts = {0: dense_phase(0, xas.pop(0))}
            qt_setup()
            qa_setup()
            prev_exp = scores_exp_phase(cts.pop(0))
            if n_it > 1:
                cts[1] = dense_phase(1, xas.pop(1))
            for it in range(1, n_it + 1):
                if it < n_it:
                    if it + 2 < n_it:
                        xas[it + 2] = load_xa01(it + 2)
                    if it + 1 < n_it:
                        cts[it + 1] = dense_phase(it + 1, xas.pop(it + 1))
                    back_phase(it - 1, prev_exp)
                    prev_exp = scores_exp_phase(cts.pop(it))
                else:
                    back_phase(it - 1, prev_exp)

    return nc


def _get_nc(masked=False):
    key = ("nc", masked)
    if key not in _COMPILED:
        nc = _build_kernel(masked=masked)
        nc.compile()
        nc.finalize()
        _COMPILED[key] = nc
    return _COMPILED[key]


def make_blobs(W, b, question_sequence_i, question_mask_i):
    """Pack per-core constants into the fp16 and fp32 setup blobs."""
    wTp = np.zeros((EA, H), np.float16)
    wTp[:E] = W.astype(np.float16).T
    wTp[E] = b.astype(np.float16)          # bias rides the aug row
    bw = np.zeros((128, BW_F), np.float16)
    bw[:, 0:128] = wTp[0:128]
    bw[:, 128:256] = wTp[128:256]
    bw[0:K2P, 256:384] = wTp[256:EA]

    qTp = np.zeros((EA, QST), np.float16)
    qTp[:E] = question_sequence_i.astype(np.float16).T
    qTp[E] = 1.0                           # ones row pairs with the bias row
    bq = np.zeros((128, BQ_F), np.float16)
    bq[:, 0:512] = qTp[0:128]
    bq[:, 512:1024] = qTp[128:256]
    bq[0:K2P, 1024:1536] = qTp[256:EA]

    b32 = np.zeros((128, BLOB32_F), np.float32)
    b32[:, OFF_B] = b.astype(np.float32)
    mb = np.where(question_mask_i == 0, np.float32(-1e30), np.float32(0.0))
    b32[:, OFF_MB : OFF_MB + 4] = mb.reshape(4, 128).T
    return bw, bq, b32


def make_in_maps(context_sequence, question_sequence, question_mask, W, b):
    in_maps = []
    for i in range(N_CORES):
        xcT = np.empty((EA, CTX), np.float16)
        xcT[:E] = context_sequence[i].T.astype(np.float16)
        xcT[E] = 1.0                       # ones row pairs with the bias row
        # partition-major tiling with >=2KB contiguous per-partition rows
        xk01 = np.ascontiguousarray(
            xcT[0:256].reshape(2, 128, N_IT, ST2).transpose(2, 1, 0, 3))
        xk2 = np.ascontiguousarray(xcT[256:EA])
        bw, bq, b32 = make_blobs(W, b, question_sequence[i], question_mask[i])
        in_maps.append({"xk01": xk01, "xk2": xk2, "bw": bw, "bq": bq,
                        "b32": b32})
    return in_maps


def assemble_out(res):
    outs = []
    for i in range(N_CORES):
        o4 = res.results[i]["out4"]        # [it, p, ci, h] fp16
        outs.append(o4.transpose(0, 2, 1, 3).reshape(CTX, H))
    return np.stack(outs, axis=0).astype(np.float32)


def kernel(context_sequence, question_sequence, question_mask, W, b):
    from concourse.bass_utils import run_bass_kernel_spmd

    masked = bool(np.any(np.asarray(question_mask) == 0))
    nc = _get_nc(masked=masked)
    in_maps = make_in_maps(
        context_sequence, question_sequence, question_mask, W, b)
    res = run_bass_kernel_spmd(nc, in_maps, core_ids=list(range(N_CORES)))
    return assemble_out(res)


# revision 26
# speedup vs baseline: 1.1097x; 1.0981x over previous
"""Trainium2 Bass kernel for AlignedQuestionEmbeddingLayer.

Computation (per batch element):
    C = relu(Xc @ W.T + b)            # [4096, 128]
    Q = relu(Xq @ W.T + b)            # [512, 128]
    S = C @ Q.T  (+ mask)             # [4096, 512]
    A = softmax(S, axis=-1)
    out = A @ Q                       # [4096, 128]

Sharding: data-parallel over batch, one batch element per NeuronCore (8 cores).

Device-side design notes (v2):
  - Dense contracts over E=300 (+1 bias row) = chunks of 128/128/45 on the
    partition axis; the 45-row chunk is NOT zero-padded to 128, cutting
    input DMA bytes by ~21%. Row 300 of xT is all-ones and row 300 of the
    W blob holds b, so x@W.T+b comes straight out of the matmul.
  - Inputs are pre-transposed and fp16-cast on host (fp16 matmuls are 4x
    fp32 on the PE; no on-device transposes; half the DMA bytes).
  - The main loop runs 4 iterations of 1024 context rows (2x the old
    super-tile): half the DMA triggers (the Sync sequencer's ~0.6us
    serial cost per trigger), half the loop-carried semaphores.
  - bw/bq/xk2 setup DMAs issue on the Scalar (ACT) HWDGE queue in
    parallel with the x loads on Sync, so the question dense no longer
    waits ~7us for its operands (the old kernel lost ~2us + a HAM
    re-throttle there).
  - Scores are computed transposed ([q partitions, c free]) so the final
    matmul (lhsT=expS_T chunk, rhs=[Q|ones] in bf16) produces output rows
    and softmax denominators in one PSUM accumulation.
  - Softmax skips max-subtraction: scores are bounded (~|s|<40) so exp()
    is safe in fp32; exp is stored bf16 (fp16 lacks the range).
  - Output is stored fp16 (halves store traffic; |out| <= max|Q| so fp16
    is safe, ~5e-4 rel err) and upcast to fp32 on host.
  - The loop is software-pipelined with lag 1 (back/norm/store of it-1
    between dense and scores of it) and x loads run 2 iterations ahead.
  - A short burst of dummy matmuls at kernel start warms the PE HAM
    clock gate during the setup-DMA window (2.4 GHz vs 1.2 cold).
"""

import sys

import numpy as np

sys.path.insert(0, "/opt/trn_rl_repo")

B, CTX, QST, E, H = 8, 4096, 512, 300, 128
N_CORES = 8
EA = E + 1          # augmented row E carries the bias / ones
K2P = EA - 256      # 45 partitions in the third contraction chunk
ST2 = 1024          # context rows per main-loop iteration
N_IT = CTX // ST2   # 4 iterations
N_WARM = 6          # dummy matmuls: ~3.6us contiguous PE busy flips the HAM
                    # clock gate to 2.4GHz (needs a fully-busy 3.4us window)

# fp16 blobs: W chunks (with bias row) and XqT chunks (with ones row)
BW_F = 3 * 128            # bw: [:, k*128 : +128], k=0..2 (k=2 rows 0:45)
BQ_F = 3 * 512            # bq: [:, k*512 : +512], k=0..2 (k=2 rows 0:45)
# fp32 blob: bias column + 4 maskbias columns (only read by masked variant)
OFF_B = 0
OFF_MB = 1
BLOB32_F = 5

_COMPILED = {}


def _build_kernel(n_it=N_IT, masked=False):
    import concourse.bass as bass
    import concourse.tile as tile
    from concourse import bacc, mybir

    f32 = mybir.dt.float32
    f16 = mybir.dt.float16
    bf16 = mybir.dt.bfloat16
    AF = mybir.ActivationFunctionType
    MAX = mybir.AluOpType.max

    nc = bacc.Bacc(
        "TRN2", target_bir_lowering=False, debug=False, num_devices=N_CORES
    )

    xk01_d = nc.declare_dram_parameter("xk01", [N_IT, 128, 2, ST2], f16,
                                       isOutput=False)
    xk2_d = nc.declare_dram_parameter("xk2", [K2P, CTX], f16, isOutput=False)
    bw_d = nc.declare_dram_parameter("bw", [128, BW_F], f16, isOutput=False)
    bq_d = nc.declare_dram_parameter("bq", [128, BQ_F], f16, isOutput=False)

    b32_d = nc.declare_dram_parameter("b32", [128, BLOB32_F], f32, isOutput=False)
    out_d = nc.declare_dram_parameter("out4", [N_IT, 128, ST2 // 128, H], f16,
                                      isOutput=True)

    with tile.TileContext(nc) as tc:
        with (
            tc.tile_pool(name="const", bufs=1) as const_pool,
            # bufs=2 is deliberate DMA staging: the 3rd x load's trigger
            # blocks on slot-free, so it doesn't steal HBM bandwidth from
            # the critical first-wave blobs (bq/bw/x0) on the single queue
            tc.tile_pool(name="xin", bufs=2) as xin_pool,
            tc.tile_pool(name="xk2in", bufs=2) as xk2_pool,
            tc.tile_pool(name="ct", bufs=2) as ct_pool,
            tc.tile_pool(name="exps", bufs=2) as exps_pool,
            tc.tile_pool(name="outs", bufs=2) as outs_pool,
            tc.tile_pool(name="pct", bufs=1, space=bass.MemorySpace.PSUM) as pct_pool,
            tc.tile_pool(name="pst", bufs=2, space=bass.MemorySpace.PSUM) as pst_pool,
            tc.tile_pool(name="po", bufs=2, space=bass.MemorySpace.PSUM) as po_pool,
        ):
            # ---- setup DMAs first.  bq leads (longest dependency chain:
            # qt -> scores -> exp); x loads on Sync in parallel.  Later x
            # loads stage behind pool back-pressure (see xin bufs=2). ----
            def load_x(it, split=False):
                xa = xin_pool.tile([128, 2, ST2], f16, tag="xa")
                xk2t = xk2_pool.tile([K2P, ST2], f16, tag="xk2")
                if split:
                    for k in range(2):
                        nc.sync.dma_start(xa[:, k, :], xk01_d[it, :, k, :])
                else:
                    nc.sync.dma_start(xa[:], xk01_d[it])
                # on Sync, not Scalar: a staged trigger parked on the ACT
                # queue would head-block the exp instructions behind it
                nc.sync.dma_start(xk2t[:], xk2_d[:, it * ST2 : (it + 1) * ST2])
                return xa, xk2t

            bq_sb = const_pool.tile([128, BQ_F], f16, tag="bq")
            nc.scalar.dma_start(bq_sb[:], bq_d[:])
            xas = {0: load_x(0, split=True)}
            bw_sb = const_pool.tile([128, BW_F], f16, tag="bw")
            nc.scalar.dma_start(bw_sb[:], bw_d[:])
            if masked:
                setup32 = const_pool.tile([128, BLOB32_F], f32, tag="setup32")
                nc.scalar.dma_start(setup32[:], b32_d[:])
            for _it in range(1, min(3, n_it)):
                xas[_it] = load_x(_it)

            # ---- PE warmup: matmuls on an uninitialized tile, results
            # discarded (next dense's start=True clears the bank) ----------
            warm = const_pool.tile([128, 512], f16, tag="warm")
            nc.gpsimd.memset(warm[:], 0.0)
            warm_ps = pct_pool.tile([H, ST2], f32, tag="pct")
            for _ in range(N_WARM):
                nc.tensor.matmul(
                    warm_ps[:, 0:512], warm[:, 0:128], warm[:], start=True,
                    stop=True, skip_group_check=True,
                )

            def w_chunk(k):  # [*, 128] fp16 W.T chunk (row E holds b)
                if k < 2:
                    return bw_sb[:, k * 128 : (k + 1) * 128]
                return bw_sb[0:K2P, 256:384]

            def q_chunk(k):  # [*, 512] fp16 Xq.T chunk (row E all-ones)
                if k < 2:
                    return bq_sb[:, k * 512 : (k + 1) * 512]
                return bq_sb[0:K2P, 1024:1536]

            def x_chunk(xpair, it, k):
                if k < 2:
                    return xpair[0][:, k, :]
                return xpair[1][:]

            # question dense twice (bias rides the aug row):
            #   QT [h, q] fp16 for the scores lhsT
            #   Q  [q, h] bf16 (as [Q|ones] per chunk) for the final rhs
            qt_sb = const_pool.tile([H, QST], f16, tag="qt")
            qa_sb = []
            for j in range(4):
                qa_tile = const_pool.tile([128, H + 1], bf16, tag=f"qa{j}")
                qa_sb.append(qa_tile)

            def qt_setup():
                psum_q = pst_pool.tile([128, ST2], f32, tag="pst")
                for k in range(3):
                    nc.tensor.matmul(
                        psum_q[0:H, 0:QST], w_chunk(k), q_chunk(k),
                        start=(k == 0), stop=(k == 2),
                    )
                nc.vector.tensor_scalar(qt_sb[:], psum_q[0:H, 0:QST],
                                        0.0, None, MAX)

            def qa_setup():
                psum_qd = pst_pool.tile([128, ST2], f32, tag="pst")
                for j in range(4):
                    for k in range(3):
                        nc.tensor.matmul(
                            psum_qd[:, j * 128 : (j + 1) * 128],
                            q_chunk(k)[:, j * 128 : (j + 1) * 128], w_chunk(k),
                            start=(k == 0), stop=(k == 2),
                        )
                for j in range(4):
                    nc.vector.tensor_scalar(
                        qa_sb[j][:, 0:H], psum_qd[:, j * 128 : (j + 1) * 128],
                        0.0, None, MAX,
                    )
                    nc.gpsimd.memset(qa_sb[j][:, H : H + 1], 1.0)

            # ---- software-pipelined main loop -----------------------------
            def dense_phase(it, xa):
                # one matmul may only target a single PSUM bank (512 fp32
                # free elements), so each k-chunk runs as two halves
                psum_ct = pct_pool.tile([128, ST2], f32, tag="pct")
                for h in range(2):
                    cols = slice(h * 512, (h + 1) * 512)
                    for k in range(3):
                        nc.tensor.matmul(
                            psum_ct[0:H, cols], w_chunk(k),
                            x_chunk(xa, it, k)[:, cols],
                            start=(k == 0), stop=(k == 2),
                        )
                ct_sb = ct_pool.tile([H, ST2], f16, tag="ct")
                for h in range(2):  # halves so scores can start on half 0
                    cols = slice(h * 512, (h + 1) * 512)
                    nc.vector.tensor_scalar(ct_sb[:, cols], psum_ct[0:H, cols],
                                            0.0, None, MAX)
                return ct_sb

            def scores_exp_phase(ct_sb):
                es = exps_pool.tile([128, 4 * ST2], bf16, tag="es")
                for j in range(4):
                    ps = pst_pool.tile([128, ST2], f32, tag="pst")
                    for h in range(2):
                        cols = slice(h * 512, (h + 1) * 512)
                        nc.tensor.matmul(
                            ps[:, cols], qt_sb[:, j * 128 : (j + 1) * 128],
                            ct_sb[:, cols], start=True, stop=True,
                        )
                    eslice = es[:, j * ST2 : (j + 1) * ST2]
                    if masked:
                        nc.scalar.activation(
                            eslice, ps[:], AF.Exp,
                            bias=setup32[:, OFF_MB + j : OFF_MB + j + 1],
                        )
                    else:
                        nc.scalar.activation(eslice, ps[:], AF.Exp)
                return es

            def back_phase(it, es):
                def exp_chunk(ci, j):  # [128 q, 128 c] bf16 lhsT
                    return es[:, j * ST2 + ci * 128 : j * ST2 + (ci + 1) * 128]

                split_store = it == n_it - 1  # shorter kernel tail
                o_big = outs_pool.tile([128, ST2 // 128, H], f16, tag="obig")
                for cp in range(ST2 // 256):  # ci pairs share one PSUM bank
                    po2 = po_pool.tile([128, 2, H + 1], f32, tag="po")
                    for ch in range(2):
                        ci = 2 * cp + ch
                        for j in range(4):
                            nc.tensor.matmul(
                                po2[:, ch, :], exp_chunk(ci, j), qa_sb[j][:],
                                start=(j == 0), stop=(j == 3),
                            )
                    recip2 = outs_pool.tile([128, 2], f32, tag="recip")
                    sums2 = po2[:, :, H : H + 1].rearrange("p a b -> p (a b)")
                    nc.vector.reciprocal(recip2[:], sums2)
                    for ch in range(2):
                        ci = 2 * cp + ch
                        nc.vector.tensor_scalar_mul(
                            o_big[:, ci, :], po2[:, ch, 0:H],
                            recip2[:, ch : ch + 1])
                    if split_store and cp % 2 == 1:
                        nc.sync.dma_start(
                            out_d[it, :, 2 * cp - 2 : 2 * cp + 2, :],
                            o_big[:, 2 * cp - 2 : 2 * cp + 2, :])
                if not split_store:
                    nc.sync.dma_start(out_d[it], o_big[:])

            # prologue: dense+scores of it 0 as soon as x0 arrives; scores 0
            # goes before dense 1 so the exp chain on ACT starts earliest
            cts = {0: dense_phase(0, xas.pop(0))}
            qt_setup()
            qa_setup()
            prev_exp = scores_exp_phase(cts.pop(0))
            if n_it > 1:
                cts[1] = dense_phase(1, xas.pop(1))
            for it in range(1, n_it + 1):
                if it < n_it:
                    if it + 2 < n_it:
                        xas[it + 2] = load_x(it + 2)
                    if it + 1 < n_it:
                        cts[it + 1] = dense_phase(it + 1, xas.pop(it + 1))
                    back_phase(it - 1, prev_exp)
                    prev_exp = scores_exp_phase(cts.pop(it))
                else:
                    back_phase(it - 1, prev_exp)

    return nc


def _get_nc(masked=False):
    key = ("nc", masked)
    if key not in _COMPILED:
        nc = _build_kernel(masked=masked)
        nc.compile()
        nc.finalize()
        _COMPILED[key] = nc
    return _COMPILED[key]


def make_blobs(W, b, question_sequence_i, question_mask_i):
    """Pack per-core constants into the fp16 and fp32 setup blobs."""
    wTp = np.zeros((EA, H), np.float16)
    wTp[:E] = W.astype(np.float16).T
    wTp[E] = b.astype(np.float16)          # bias rides the aug row
    bw = np.zeros((128, BW_F), np.float16)
    bw[:, 0:128] = wTp[0:128]
    bw[:, 128:256] = wTp[128:256]
    bw[0:K2P, 256:384] = wTp[256:EA]

    qTp = np.zeros((EA, QST), np.float16)
    qTp[:E] = question_sequence_i.astype(np.float16).T
    qTp[E] = 1.0                           # ones row pairs with the bias row
    bq = np.zeros((128, BQ_F), np.float16)
    bq[:, 0:512] = qTp[0:128]
    bq[:, 512:1024] = qTp[128:256]
    bq[0:K2P, 1024:1536] = qTp[256:EA]

    b32 = np.zeros((128, BLOB32_F), np.float32)
    b32[:, OFF_B] = b.astype(np.float32)
    mb = np.where(question_mask_i == 0, np.float32(-1e30), np.float32(0.0))
    b32[:, OFF_MB : OFF_MB + 4] = mb.reshape(4, 128).T
    return bw, bq, b32


def make_in_maps(context_sequence, question_sequence, question_mask, W, b):
    in_maps = []
    for i in range(N_CORES):
        xcT = np.empty((EA, CTX), np.float16)
        xcT[:E] = context_sequence[i].T.astype(np.float16)
        xcT[E] = 1.0                       # ones row pairs with the bias row
        # partition-major tiling with >=2KB contiguous per-partition rows
        xk01 = np.ascontiguousarray(
            xcT[0:256].reshape(2, 128, N_IT, ST2).transpose(2, 1, 0, 3))
        xk2 = np.ascontiguousarray(xcT[256:EA])
        bw, bq, b32 = make_blobs(W, b, question_sequence[i], question_mask[i])
        in_maps.append({"xk01": xk01, "xk2": xk2, "bw": bw, "bq": bq,
                        "b32": b32})
    return in_maps


def assemble_out(res):
    outs = []
    for i in range(N_CORES):
        o4 = res.results[i]["out4"]        # [it, p, ci, h] fp16
        outs.append(o4.transpose(0, 2, 1, 3).reshape(CTX, H))
    return np.stack(outs, axis=0).astype(np.float32)


def kernel(context_sequence, question_sequence, question_mask, W, b):
    from concourse.bass_utils import run_bass_kernel_spmd

    masked = bool(np.any(np.asarray(question_mask) == 0))
    nc = _get_nc(masked=masked)
    in_maps = make_in_maps(
        context_sequence, question_sequence, question_mask, W, b)
    res = run_bass_kernel_spmd(nc, in_maps, core_ids=list(range(N_CORES)))
    return assemble_out(res)
